# revision 5
# baseline (speedup 1.0000x reference)
"""Trainium2 Bass kernel for per-pixel dynamic-weight 3x3 aggregation.

Computation (per sample):
    out[c, h, w] = sum_{kh,kw} xpad[c, h+kh, w+kw] * weight[c % WC, kh*3+kw, h, w]
with reflect padding (pad=1) of x.

Sharding: data-parallel over batch N=8 -> one sample per NeuronCore (8 cores).

v2 design (vs the f32 baseline):
  - f16 end-to-end: host casts x/w to f16, device loads/stores f16, host casts
    the f16 output back to f32. Halves HBM traffic (DMA ~79us vs ~157us).
  - The +-1 column shifts of the 3x3 taps are folded into the *weight* DMA
    source offsets (flat shift by (1-kw) elements), so every DVE multiply
    reads both operands at column-0-aligned bases -> guaranteed 2x mode.
    The PE identity-matmul accumulation applies the shift back via column
    windows (out[f + 1-kw] += p[f]), split per PSUM bank.
  - Weight slots that the shift fills with out-of-row garbage are zeroed
    (memset) so the window accumulation adds exact zeros at row seams.
  - The two reflect-column terms (out col 0 needs x[.,1]*w_{kh,0}[.,0]; col
    127 needs x[.,126]*w_{kh,2}[.,127]) are computed from a tiny host-packed
    "aux" tensor (a repack of 2 weight columns) and injected into the kw=1
    product tile's cols 0/127 before PE consumes it.
  - No shifted x copy (the old xm) -> ACT only evacuates PSUM (f32->f16).
  - All DMA on HWDGE (sync engine): no f32->f16 cast in DMA needed anymore,
    and GpSimd stays empty (its tensor ops serialize with DVE on real HW).

Engine budget per core (cost-model): DVE ~162us (9 taps x 16 phases of
tensor_mul at 2x + tiny fixups) = bottleneck; PE ~123us; DMA ~79us; ACT ~27us.

Partition mapping: p = q*32 + wc (q = row-quarter of the chunk, wc = weight
channel). Free dims = (g, row, col), channel c = g*32 + wc.
"""

import numpy as np

import concourse.tile as tile
from concourse import bacc, mybir
from concourse.ap import AP
from concourse.bass_utils import run_bass_kernel_spmd

# Problem constants (hardcoded per contract).
N, C, H, W = 8, 256, 128, 128
WC, KK = 32, 9
G = C // WC  # 8 channel groups share one weight channel
NCORES = 8

R = 32            # rows per chunk
NCHUNK = H // R   # 4
Q = R // 4        # 8 rows handled per partition (one quarter of a chunk)
XROWS = Q + 2     # rows in the x tiles (1-row halo on each side)

FP32 = mybir.dt.float32
F16 = mybir.dt.float16

HW_ = H * W            # channel stride in x/out (elements)
WC_STRIDE = KK * HW_   # wc stride in weight
QW = Q * W

_compiled = None


def _dram_ap(t, offset, dims):
    """AP over a DRAM tensor with explicit [stride, count] dims (elements)."""
    return AP(tensor=t.ap().tensor, offset=int(offset), ap=[[int(s), int(c)] for s, c in dims])


def _sb_ap(base, offset, dims):
    """AP over an SBUF tile: keep its partition dim, custom free dims."""
    return AP(
        tensor=base.tensor,
        offset=base.offset + int(offset),
        ap=[list(base.ap[0])] + [[int(s), int(c)] for s, c in dims],
    )


def build(reps: int = 1, do_dma: bool = True, do_compute: bool = True):
    nc = bacc.Bacc("TRN2", target_bir_lowering=False, debug=False, num_devices=1)

    x_t = nc.dram_tensor("x", [C, H, W], F16, kind="ExternalInput")
    w_t = nc.dram_tensor("w", [WC, KK, H, W], F16, kind="ExternalInput")
    aux_t = nc.dram_tensor("aux", [NCHUNK, 4, WC, 3, Q, 2], F16, kind="ExternalInput")
    id_t = nc.dram_tensor("ident", [128, 128], F16, kind="ExternalInput")
    o_t = nc.dram_tensor("out", [C, H, W], F16, kind="ExternalOutput")

    with tile.TileContext(nc) as tc:
        with (
            tc.tile_pool(name="const", bufs=1) as const_pool,
            tc.tile_pool(name="xe", bufs=2) as xe_pool,
            tc.tile_pool(name="wp", bufs=2) as w_pool,
            tc.tile_pool(name="wfx", bufs=2) as wfx_pool,
            tc.tile_pool(name="prod", bufs=2) as prod_pool,
            tc.tile_pool(name="tmp", bufs=2) as tmp_pool,
            tc.tile_pool(name="osb", bufs=3) as out_pool,
            tc.tile_pool(name="ps", bufs=2, space="PSUM") as psum_pool,
        ):
            ident = const_pool.tile([128, 128], F16)
            nc.sync.dma_start(ident[:], id_t.ap())

            def load_chunk(ch):
                r0 = ch * R
                xe = xe_pool.tile([128, G, XROWS, W], F16, tag="xe")
                wt = w_pool.tile([128, KK, Q, W], F16, tag="wt")
                wfx = wfx_pool.tile([128, 3, Q, 2], F16, tag="wfx")

                if do_dma:
                    # x: per-q [32, G, rows, W] f16, rows r0+Qq-1+t
                    for q in range(4):
                        t0 = 1 if (ch == 0 and q == 0) else 0
                        t1 = XROWS - 2 if (ch == NCHUNK - 1 and q == 3) else XROWS - 1
                        nrow = t1 - t0 + 1
                        src = _dram_ap(
                            x_t,
                            (r0 + Q * q - 1 + t0) * W,
                            [(HW_, WC), (32 * HW_, G), (1, nrow * W)],
                        )
                        nc.sync.dma_start(
                            xe[32 * q : 32 * (q + 1), :, t0 : t1 + 1, :], src
                        )
                    if ch == 0:  # reflect top: row -1 -> row 1
                        src = _dram_ap(x_t, 1 * W, [(HW_, WC), (32 * HW_, G), (1, W)])
                        nc.sync.dma_start(xe[0:32, :, 0:1, :], src)
                    if ch == NCHUNK - 1:  # reflect bottom: row 128 -> 126
                        src = _dram_ap(
                            x_t, (H - 2) * W, [(HW_, WC), (32 * HW_, G), (1, W)]
                        )
                        nc.sync.dma_start(xe[96:128, :, XROWS - 1 : XROWS, :], src)

                    # w: per-q, per-kw group {kw, kw+3, kw+6}, source shifted
                    # by (1-kw) elements (the column shift of the tap)
                    for q in range(4):
                        for kw in range(3):
                            src = _dram_ap(
                                w_t,
                                kw * HW_ + (r0 + Q * q) * W + (1 - kw),
                                [(WC_STRIDE, WC), (3 * HW_, 3), (1, QW)],
                            )
                            dst = _sb_ap(
                                wt[32 * q : 32 * (q + 1)],
                                kw * QW,
                                [(3 * QW, 3), (1, QW)],
                            )
                            nc.sync.dma_start(dst, src)

                    # aux -> wfx: [128, 48] contiguous per chunk
                    src = _dram_ap(aux_t, ch * 128 * 48, [(48, 128), (1, 48)])
                    nc.sync.dma_start(
                        wfx[:].rearrange("p a b c -> p (a b c)"), src
                    )

                if do_compute:
                    # zero the row-seam garbage the shifts dragged in:
                    # kw=0 taps {0,3,6}: col 127 slots; kw=2 taps {2,5,8}: col 0
                    for k in (0, 3, 6):
                        nc.vector.memset(wt[:, k : k + 1, :, 127:128], 0)
                    for k in (2, 5, 8):
                        nc.vector.memset(wt[:, k : k + 1, :, 0:1], 0)
                return xe, wt, wfx

            def run_chunk(ch, tiles):
                r0 = ch * R
                xe, wt, wfx = tiles
                for ph in range(4):  # g-pair phases: g in {2ph, 2ph+1}
                    pkw0 = prod_pool.tile([128, 3, 2, Q, W], F16, tag="pkw0")
                    pkw1 = prod_pool.tile([128, 3, 2, Q, W], F16, tag="pkw1")
                    pkw2 = prod_pool.tile([128, 3, 2, Q, W], F16, tag="pkw2")
                    pkw = [pkw0, pkw1, pkw2]
                    if do_compute:
                        # 3 mega multiplies: all kh for one kw in one DVE op
                        for kw in range(3):
                            xin = _sb_ap(
                                xe[:],
                                2 * ph * XROWS * W,
                                [(W, 3), (XROWS * W, 2), (W, Q), (1, W)],
                            )
                            win = _sb_ap(
                                wt[:],
                                kw * QW,
                                [(3 * QW, 3), (0, 2), (W, Q), (1, W)],
                            )
                            nc.vector.tensor_mul(pkw[kw][:], xin, win)

                        # reflect-column fixup: tmp[side, kh, g, r] =
                        #   xe[g, r+kh, col(side)] * wfx[kh, r, side]
                        tmpt = tmp_pool.tile([128, 2, 3, 2, Q], F16, tag="tmp")
                        for side, col in ((0, 1), (1, W - 2)):
                            xfix = _sb_ap(
                                xe[:],
                                2 * ph * XROWS * W + col,
                                [(W, 3), (XROWS * W, 2), (W, Q)],
                            )
                            wfig = _sb_ap(
                                wfx[:], side, [(2 * Q, 3), (0, 2), (2, Q)]
                            )
                            nc.vector.tensor_mul(tmpt[:, side], xfix, wfig)
                        # inject into the kw=1 product's cols 0 / 127
                        for side, col in ((0, 0), (1, W - 1)):
                            pslice = _sb_ap(
                                pkw[1][:],
                                col,
                                [(2 * QW, 3), (QW, 2), (W, Q)],
                            )
                            nc.vector.scalar_tensor_tensor(
                                pslice, pslice, 1.0, tmpt[:, side],
                                mybir.AluOpType.mult, mybir.AluOpType.add,
                            )

                    pst = psum_pool.tile([128, 2048], FP32)
                    if do_compute:
                        # PE tap-sum: per PSUM bank, windowed identity matmuls
                        # out[f + (1-kw)] += p[f]
                        for b in range(4):
                            n_in_bank = 0
                            for kw, khi in (
                                (1, 0), (1, 1), (1, 2),
                                (0, 0), (0, 1), (0, 2),
                                (2, 0), (2, 1), (2, 2),
                            ):
                                s = 1 - kw
                                j0 = max(512 * b, s) if s > 0 else 512 * b
                                j1 = min(512 * b + 512, 2048 + min(s, 0))
                                pflat = pkw[kw][:, khi].rearrange(
                                    "p g r c -> p (g r c)"
                                )
                                nc.tensor.matmul(
                                    pst[:, j0:j1],
                                    ident[:],
                                    pflat[:, j0 - s : j1 - s],
                                    start=(n_in_bank == 0),
                                    stop=(n_in_bank == 8),
                                )
                                n_in_bank += 1
                    osb = out_pool.tile([128, 2048], F16)
                    if do_compute:
                        nc.scalar.copy(osb[:], pst[:])
                    for q in range(4 if do_dma else 0):
                        dst = _dram_ap(
                            o_t,
                            2 * ph * 32 * HW_ + (r0 + Q * q) * W,
                            [(HW_, WC), (32 * HW_, 2), (1, QW)],
                        )
                        nc.sync.dma_start(dst, osb[32 * q : 32 * (q + 1), :])

            def emit_body():
                # software-pipelined: prefetch chunk ch+1 before computing ch
                tiles = load_chunk(0)
                for ch in range(NCHUNK):
                    nxt = load_chunk(ch + 1) if ch + 1 < NCHUNK else None
                    run_chunk(ch, tiles)
                    tiles = nxt

            if reps == 1:
                emit_body()
            else:  # timing builds: repeat the whole kernel on-device
                with tc.For_i(
                    0, reps, 1,
                    hint_engines=(mybir.EngineType.PE, mybir.EngineType.DVE),
                ):
                    emit_body()

    nc.compile()
    return nc


def _get_compiled():
    global _compiled
    if _compiled is None:
        _compiled = build()
    return _compiled


def make_core_inputs(x_i: np.ndarray, w_i: np.ndarray) -> dict:
    """Host-side packing for one sample: f16 casts + the aux edge-weight
    repack (w columns 0 of the kw=0 taps and 127 of the kw=2 taps, laid out
    [ch, q, wc, kh, r, side] so each partition's 48 values are contiguous)."""
    x16 = np.ascontiguousarray(x_i, dtype=np.float16)
    w16 = np.ascontiguousarray(w_i, dtype=np.float16)
    w0 = w16[:, 0::3, :, 0]      # [wc, kh, h] = w[wc, 3kh+0, h, 0]
    w127 = w16[:, 2::3, :, 127]  # [wc, kh, h] = w[wc, 3kh+2, h, 127]
    aux = np.stack([w0, w127], axis=-1)          # [wc, kh, h, side]
    aux = aux.reshape(WC, 3, NCHUNK, 4, Q, 2)    # h -> (ch, q, r)
    aux = np.ascontiguousarray(aux.transpose(2, 3, 0, 1, 4, 5))
    return {
        "x": x16,
        "w": w16,
        "aux": aux,
        "ident": np.eye(128, dtype=np.float16),
    }


def kernel(x: np.ndarray, weight: np.ndarray) -> np.ndarray:
    nc = _get_compiled()
    in_maps = [make_core_inputs(x[i], weight[i]) for i in range(NCORES)]
    res = run_bass_kernel_spmd(nc, in_maps, core_ids=list(range(NCORES)))
    return np.stack(
        [res.results[i]["out"].astype(np.float32) for i in range(NCORES)], axis=0
    )
